# revision 17
# baseline (speedup 1.0000x reference)
"""AdaptiveConv2d (pitch-dependent 3x3 1x1-conv) on 8 TRN2 NeuronCores.

Strategy (data-parallel, batch b -> core b):
  out[b,o,h,w] = bias[o] + sum_{i,j in 3x3} W_ij[o,c] * x[b,c, r_i(h; a(w)), c_j(w; e(w))]
  with a=dh, e=dw in {1,2,3} per (b,w).

Host ships two DATA-INDEPENDENT lookup tables per core (pure layout prep):
  tabA row (w,e)   = [ xslab(|w-e|) | xslab(w) ]          (HP=86 rows x 128 ch, bf16)
  tabB row (w,e,a) = [ xslab(wR)    | xslab(wR) shifted 2a ]
where xslab is the (HP,C) column slab with 3 reflect-pad rows before and 3
zero rows after. The DEVICE does all data-dependent work: dma_gather
(transpose mode) selects 800 rows per table using indices derived from
dh/dw, sorted by a-value into 3 blocks so the +-a row shifts are constant
AP offsets per block; then 5 K=128 matmul passes (2 taps per pass via the
stacked halves) accumulate all 9 taps in PSUM; small extra matmuls fix the
top-edge clamp rows; bias is fused into the PSUM evacuation; bf16 out is
unpermuted on host.
"""

import os
import sys
import math
import numpy as np

sys.path.insert(0, "/opt/trn_rl_repo")

import ml_dtypes  # noqa: E402

BF16 = ml_dtypes.bfloat16

B, C, O, H, W = 8, 64, 64, 80, 800
HP, PADR = 86, 3           # slab rows: 3 reflect + 80 + 3 zeros
ES = HP * 128              # elements per table row (bf16)
KW = 6                     # output columns per matmul window (N = 80*KW = 480)
NCORES = 8

_GRAPH_CACHE = {}


# --------------------------------------------------------------------------
# host-side table / index construction
# --------------------------------------------------------------------------

def _build_tables(xb):
    """xb (C,H,W) f32 -> tabA (W*3, ES) bf16, tabB (W*9, ES) bf16."""
    xt = np.ascontiguousarray(xb.transpose(2, 1, 0))          # (W,H,C)
    xpad = np.zeros((W, HP, C), np.float32)
    xpad[:, PADR:PADR + H] = xt
    xpad[:, 0] = xt[:, 3]
    xpad[:, 1] = xt[:, 2]
    xpad[:, 2] = xt[:, 1]
    w = np.arange(W)
    tabA = np.zeros((W, 3, HP, 128), BF16)
    tabB = np.zeros((W, 3, 3, HP, 128), BF16)
    for e in (1, 2, 3):
        wL = np.abs(w - e)
        wR = np.where(w + e < W, w + e, 2 * W - 1 - w - e)
        tabA[:, e - 1, :, :64] = xpad[wL]
        tabA[:, e - 1, :, 64:] = xpad
        lowR = xpad[wR].astype(BF16)
        for a in (1, 2, 3):
            tabB[:, e - 1, a - 1, :, :64] = lowR
            up = np.zeros((W, HP, C), np.float32)
            lo_hp = max(0, PADR - 2 * a)
            hi_hp = PADR + H - 2 * a                           # exclusive
            up[:, lo_hp:hi_hp] = xt[wR][:, lo_hp - PADR + 2 * a: hi_hp - PADR + 2 * a]
            tabB[:, e - 1, a - 1, :, 64:] = up
    return tabA.reshape(W * 3, ES), tabB.reshape(W * 9, ES)


def _sort_info(ab, eb):
    """Per-core sorted layout. Returns (block_cols list of arrays, nA list)."""
    order = np.argsort(ab, kind="stable")
    blocks = [order[ab[order] == a] for a in (1, 2, 3)]
    return blocks


def _pack_indices(blocks, eb, ab, caps, nchunks):
    """Build (128, nchunks*16) int16 index tensor + column map."""
    S_pad = nchunks * 128
    rA = np.zeros(S_pad, np.int16)
    rB = np.zeros(S_pad, np.int16)
    base = 0
    colmap = []                                   # (global sorted pos, source col)
    for a0, cols, cap in zip((1, 2, 3), blocks, caps):
        e = eb[cols]
        rA[base:base + len(cols)] = cols * 3 + (e - 1)
        rB[base:base + len(cols)] = cols * 9 + (e - 1) * 3 + (a0 - 1)
        colmap.append((base, cols))
        base += cap
    idx = np.zeros((128, nchunks * 16), np.int16)
    for c in range(nchunks):
        seg_a = rA[c * 128:(c + 1) * 128].reshape(8, 16)      # k = s*16+p
        seg_b = rB[c * 128:(c + 1) * 128].reshape(8, 16)
        idx[:16, c * 16:c * 16 + 8] = seg_a.T
        idx[:16, c * 16 + 8:c * 16 + 16] = seg_b.T
    # each Q7 core reads indices from its own 16 partitions -> replicate
    idx[:] = np.tile(idx[:16], (8, 1))
    return idx, colmap


def _pack_weights(weight):
    """(9,O,C) f32 -> (128, 6*64) bf16 lhsT blocks."""
    WT = weight.transpose(0, 2, 1).astype(np.float32)         # (9, C, O)
    Z = np.zeros((C, O), np.float32)
    blocks = [
        np.concatenate([WT[0], WT[1]], 0),     # pass1  @-a  : (D,L)+(D,C)
        np.concatenate([WT[3], WT[4]], 0),     # pass2  @ 0  : (C,L)+(C,C)
        np.concatenate([WT[6], WT[7]], 0),     # pass3  @+a  : (U,L)+(U,C)  (also corrA)
        np.concatenate([WT[2], WT[8]], 0),     # pass4  @-a B: (D,R)+(U,R)
        np.concatenate([WT[5], Z], 0),         # pass5  @ 0 B: (C,R)
        np.concatenate([WT[8], Z], 0),         # corrB       : (U,R) rows
    ]
    return np.concatenate(blocks, 1).astype(BF16)             # (128, 384)


# --------------------------------------------------------------------------
# device graph
# --------------------------------------------------------------------------

def _build_graph(caps, nchunks):
    key = (tuple(caps), nchunks)
    if key in _GRAPH_CACHE:
        return _GRAPH_CACHE[key]

    import concourse.bass as bass                 # noqa: F401
    import concourse.mybir as mybir
    import concourse.tile as tile
    from concourse import bacc

    S_pad = nchunks * 128
    bf16, i16, f32 = mybir.dt.bfloat16, mybir.dt.int16, mybir.dt.float32

    nc = bacc.Bacc("TRN2", target_bir_lowering=False, debug=False,
                   num_devices=NCORES, num_swdge_queues=4)
    tabA = nc.dram_tensor("tabA", [W * 3, ES], bf16, kind="ExternalInput")
    tabB = nc.dram_tensor("tabB", [W * 9, ES], bf16, kind="ExternalInput")
    idx_d = nc.dram_tensor("idx", [128, nchunks * 16], i16, kind="ExternalInput")
    wts_d = nc.dram_tensor("wts", [128, 6 * 64], bf16, kind="ExternalInput")
    bias_d = nc.dram_tensor("bias", [64, 1], f32, kind="ExternalInput")
    out_d = nc.dram_tensor("out", [nchunks, 64, H, 128], bf16,
                           kind="ExternalOutput")

    # block layout in global sorted coords
    starts = [0, caps[0], caps[0] + caps[1]]
    blocks = [(starts[i], starts[i] + caps[i], i + 1) for i in range(3)]

    with tile.TileContext(nc) as tc:
        with (
            tc.tile_pool(name="const", bufs=1) as constp,
            tc.tile_pool(name="slabA", bufs=3) as poolA,
            tc.tile_pool(name="slabB", bufs=3) as poolB,
            tc.tile_pool(name="stage", bufs=2) as stagep,
            tc.tile_pool(name="psum", bufs=6, space="PSUM") as psump,
            tc.tile_pool(name="psumc", bufs=2, space="PSUM") as psumcp,
        ):
            idx_sb = constp.tile([128, nchunks * 16], i16)
            nc.sync.dma_start(idx_sb[:], idx_d[:])
            wts_sb = constp.tile([128, 6 * 64], bf16)
            nc.sync.dma_start(wts_sb[:], wts_d[:])
            bias_sb = constp.tile([64, 1], f32)
            nc.sync.dma_start(bias_sb[:], bias_d[:])

            def lhs(p):
                return wts_sb[:, p * 64:(p + 1) * 64]

            evac_flip = [0]

            for cch in range(nchunks):
                k0g = cch * 128
                slabA = poolA.tile([128, HP, 128], bf16)
                nc.gpsimd.dma_gather(
                    out_ap=slabA[:], in_ap=tabA[:],
                    idxs_ap=idx_sb[:, cch * 16:cch * 16 + 8],
                    num_idxs=128, num_idxs_reg=128, elem_size=ES,
                    transpose=True, queue_num=(2 * cch) % 4)
                slabB = poolB.tile([128, HP, 128], bf16)
                nc.gpsimd.dma_gather(
                    out_ap=slabB[:], in_ap=tabB[:],
                    idxs_ap=idx_sb[:, cch * 16 + 8:cch * 16 + 16],
                    num_idxs=128, num_idxs_reg=128, elem_size=ES,
                    transpose=True, queue_num=(2 * cch + 1) % 4)
                stage = stagep.tile([64, H, 128], bf16)
                S_tot = caps[0] + caps[1] + caps[2]
                if k0g + 128 > S_tot:
                    nc.vector.memset(stage[:, :, max(0, S_tot - k0g):], 0.0)

                pieces = []
                for (blo, bhi, a0) in blocks:
                    lo, hi = max(blo, k0g), min(bhi, k0g + 128)
                    if lo < hi:
                        pieces.append((lo - k0g, hi - k0g, a0))

                for (lo, hi, a0) in pieces:
                    # long contiguous inner runs: (hr rows) x (cw cols) <= 512
                    cw = min(hi - lo, 120)
                    hr = max(1, min(H, 480 // cw))
                    for wlo in range(lo, hi, cw):
                        kw = min(cw, hi - wlo)
                        for h0 in range(0, H, hr):
                            hh = min(hr, H - h0)
                            ps = psump.tile([64, hh, kw], mybir.dt.float32,
                                            tag="ps")
                            pv = ps[:]

                            def rhs(slab, d):
                                r0 = PADR + d + h0
                                return slab[:, r0:r0 + hh, wlo:wlo + kw]

                            nc.tensor.matmul(pv, lhs(0), rhs(slabA, -a0),
                                             start=True, stop=False)
                            nc.tensor.matmul(pv, lhs(1), rhs(slabA, 0),
                                             start=False, stop=False)
                            nc.tensor.matmul(pv, lhs(2), rhs(slabA, +a0),
                                             start=False, stop=False)
                            nc.tensor.matmul(pv, lhs(3), rhs(slabB, -a0),
                                             start=False, stop=False)
                            nc.tensor.matmul(pv, lhs(4), rhs(slabB, 0),
                                             start=False, stop=True)
                            dst = stage[:, h0:h0 + hh, wlo:wlo + kw]
                            if evac_flip[0] % 2 == 0:
                                nc.vector.tensor_scalar_add(dst, pv,
                                                            bias_sb[:, :1])
                            else:
                                nc.scalar.activation(
                                    dst, pv,
                                    mybir.ActivationFunctionType.Identity,
                                    bias=bias_sb[:, :1])
                            evac_flip[0] += 1

                    # top-edge clamp correction rows h >= H-a0
                    n = hi - lo
                    psc = psumcp.tile([64, a0, n], mybir.dt.float32,
                                      tag="psc")
                    pcv = psc[:]
                    rows = slice(PADR + H - a0, PADR + H)
                    nc.tensor.matmul(pcv, lhs(2), slabA[:, rows, lo:hi],
                                     start=True, stop=False)
                    nc.tensor.matmul(pcv, lhs(5), slabB[:, rows, lo:hi],
                                     start=False, stop=True)
                    cdst = stage[:, H - a0:H, lo:hi]
                    nc.vector.tensor_tensor(
                        out=cdst, in0=pcv, in1=cdst,
                        op=mybir.AluOpType.add)

                nc.sync.dma_start(out_d[cch], stage[:])

    nc.compile()
    _GRAPH_CACHE[key] = nc
    return nc


def _register_ntff_hook():
    """Wire up NTFF profiling (image's antenv lacks the boot-time hook)."""
    try:
        import types
        import antenv
        if "antenv.axon_hooks" not in sys.modules:
            mod = types.ModuleType("antenv.axon_hooks")
            mod._HOOK = None

            def set_axon_ntff_profile_hook(hook):
                mod._HOOK = hook

            def get_axon_ntff_profile_hook():
                return mod._HOOK

            mod.set_axon_ntff_profile_hook = set_axon_ntff_profile_hook
            mod.get_axon_ntff_profile_hook = get_axon_ntff_profile_hook
            sys.modules["antenv.axon_hooks"] = mod
            antenv.axon_hooks = mod
        mod = sys.modules["antenv.axon_hooks"]
        if mod.get_axon_ntff_profile_hook() is None:
            from trn_agent_boot.trn_boot import _ntff_profile_via_ctypes
            hook = _ntff_profile_via_ctypes("/opt/axon/libaxon_pjrt.so")
            if hook is not None:
                mod.set_axon_ntff_profile_hook(hook)
    except Exception as e:          # profiling is best-effort
        print(f"ntff hook registration failed: {e}")


# --------------------------------------------------------------------------
# entry point
# --------------------------------------------------------------------------

def kernel(x, dh, dw, weight, bias, _sim=False, _trace=False):
    x = np.asarray(x, np.float32)
    dh = np.asarray(dh, np.float32)
    dw = np.asarray(dw, np.float32)
    weight = np.asarray(weight, np.float32)
    bias = np.asarray(bias, np.float32)

    ab_all = np.clip(dh[:, 0].astype(np.int64), 1, None)      # (B, W)
    eb_all = np.clip(dw[:, 0].astype(np.int64), 1, None)

    core_blocks = [_sort_info(ab_all[b], eb_all[b]) for b in range(B)]
    caps = [max(len(core_blocks[b][i]) for b in range(B)) for i in range(3)]
    S = sum(caps)
    nchunks = (S + 127) // 128

    nc = _build_graph(caps, nchunks)

    wts = _pack_weights(weight)
    bias_t = bias.reshape(64, 1).astype(np.float32)

    in_maps = []
    colmaps = []
    for b in range(B):
        tA, tB = _build_tables(x[b])
        idx, colmap = _pack_indices(core_blocks[b], eb_all[b], ab_all[b],
                                    caps, nchunks)
        colmaps.append(colmap)
        in_maps.append({"tabA": tA, "tabB": tB, "idx": idx,
                        "wts": wts, "bias": bias_t})

    if _sim:
        from concourse.bass_interp import CoreSim
        outs = []
        for b in range(B):
            sim = CoreSim(nc, core_id=0)
            for k, v in in_maps[b].items():
                sim.tensor(k)[:] = v
            sim.simulate()
            outs.append(np.array(sim.tensor("out")))
        results = [{"out": o} for o in outs]
        exec_ns = None
    else:
        from concourse.bass_utils import run_bass_kernel_spmd
        if _trace:
            _register_ntff_hook()
        r = run_bass_kernel_spmd(nc, in_maps, core_ids=list(range(NCORES)),
                                 trace=_trace)
        results = r.results
        exec_ns = r.exec_time_ns
        kernel.last_profile = r.profile_json

    out = np.zeros((B, O, H, W), np.float32)
    for b in range(B):
        ob = np.asarray(results[b]["out"]).astype(np.float32)
        ob = ob.transpose(1, 2, 0, 3).reshape(64, H, nchunks * 128)
        for base, cols in colmaps[b]:
            out[b][:, :, cols] = ob[:, :, base:base + len(cols)]
    kernel.last_exec_ns = exec_ns
    return out


kernel.last_exec_ns = None


# revision 22
# speedup vs baseline: 1.2733x; 1.2733x over previous
"""AdaptiveConv2d (pitch-dependent 3x3 1x1-conv) on 8 TRN2 NeuronCores.

Strategy (data-parallel, batch b -> core b):
  out[b,o,h,w] = bias[o] + sum_{i,j in 3x3} W_ij[o,c] * x[b,c, r_i(h; a(w)), c_j(w; e(w))]
  with a=dh, e=dw in {1,2,3} per (b,w).

Host ships two DATA-INDEPENDENT lookup tables per core (pure layout prep):
  tabA row (w,e)   = [ xslab(|w-e|) | xslab(w) ]          (HP=86 rows x 128 ch, bf16)
  tabB row (w,e,a) = [ xslab(wR)    | xslab(wR) shifted 2a ]
where xslab is the (HP,C) column slab with 3 reflect-pad rows before and 3
zero rows after. The DEVICE does all data-dependent work: dma_gather
(transpose mode) selects 800 rows per table using indices derived from
dh/dw, sorted by a-value into 3 blocks so the +-a row shifts are constant
AP offsets per block; then 5 K=128 matmul passes (2 taps per pass via the
stacked halves) accumulate all 9 taps in PSUM; small extra matmuls fix the
top-edge clamp rows; bias is fused into the PSUM evacuation; bf16 out is
unpermuted on host.
"""

import os
import sys
import math
import numpy as np

sys.path.insert(0, "/opt/trn_rl_repo")

import ml_dtypes  # noqa: E402

BF16 = ml_dtypes.bfloat16

B, C, O, H, W = 8, 64, 64, 80, 800
HP, PADR = 86, 3           # slab rows: 3 reflect + 80 + 3 zeros
ES = HP * 128              # elements per table row (bf16)
KW = 6                     # output columns per matmul window (N = 80*KW = 480)
NCORES = 8

_GRAPH_CACHE = {}


# --------------------------------------------------------------------------
# host-side table / index construction
# --------------------------------------------------------------------------

def _build_tables(xb):
    """xb (C,H,W) f32 -> tabA (W*3, ES) bf16, tabB (W*9, ES) bf16."""
    xt = np.ascontiguousarray(xb.transpose(2, 1, 0))          # (W,H,C)
    xpad = np.zeros((W, HP, C), np.float32)
    xpad[:, PADR:PADR + H] = xt
    xpad[:, 0] = xt[:, 3]
    xpad[:, 1] = xt[:, 2]
    xpad[:, 2] = xt[:, 1]
    w = np.arange(W)
    tabA = np.zeros((W, 3, HP, 128), BF16)
    tabB = np.zeros((W, 3, 3, HP, 128), BF16)
    for e in (1, 2, 3):
        wL = np.abs(w - e)
        wR = np.where(w + e < W, w + e, 2 * W - 1 - w - e)
        tabA[:, e - 1, :, :64] = xpad[wL]
        tabA[:, e - 1, :, 64:] = xpad
        lowR = xpad[wR].astype(BF16)
        for a in (1, 2, 3):
            tabB[:, e - 1, a - 1, :, :64] = lowR
            up = np.zeros((W, HP, C), np.float32)
            lo_hp = max(0, PADR - 2 * a)
            hi_hp = PADR + H - 2 * a                           # exclusive
            up[:, lo_hp:hi_hp] = xt[wR][:, lo_hp - PADR + 2 * a: hi_hp - PADR + 2 * a]
            tabB[:, e - 1, a - 1, :, 64:] = up
    return tabA.reshape(W * 3, ES), tabB.reshape(W * 9, ES)


def _sort_info(ab, eb):
    """Per-core sorted layout. Returns (block_cols list of arrays, nA list)."""
    order = np.argsort(ab, kind="stable")
    blocks = [order[ab[order] == a] for a in (1, 2, 3)]
    return blocks


def _pack_indices(blocks, eb, ab, caps, nchunks):
    """Build (128, nchunks*16) int16 index tensor + column map."""
    S_pad = nchunks * 128
    rA = np.zeros(S_pad, np.int16)
    rB = np.zeros(S_pad, np.int16)
    base = 0
    colmap = []                                   # (global sorted pos, source col)
    for a0, cols, cap in zip((1, 2, 3), blocks, caps):
        e = eb[cols]
        rA[base:base + len(cols)] = cols * 3 + (e - 1)
        rB[base:base + len(cols)] = cols * 9 + (e - 1) * 3 + (a0 - 1)
        colmap.append((base, cols))
        base += cap
    idx = np.zeros((128, nchunks * 16), np.int16)
    for c in range(nchunks):
        seg_a = rA[c * 128:(c + 1) * 128].reshape(8, 16)      # k = s*16+p
        seg_b = rB[c * 128:(c + 1) * 128].reshape(8, 16)
        idx[:16, c * 16:c * 16 + 8] = seg_a.T
        idx[:16, c * 16 + 8:c * 16 + 16] = seg_b.T
    # each Q7 core reads indices from its own 16 partitions -> replicate
    idx[:] = np.tile(idx[:16], (8, 1))
    return idx, colmap


def _pack_weights(weight):
    """(9,O,C) f32 -> (128, 6*64) bf16 lhsT blocks."""
    WT = weight.transpose(0, 2, 1).astype(np.float32)         # (9, C, O)
    Z = np.zeros((C, O), np.float32)
    blocks = [
        np.concatenate([WT[0], WT[1]], 0),     # pass1  @-a  : (D,L)+(D,C)
        np.concatenate([WT[3], WT[4]], 0),     # pass2  @ 0  : (C,L)+(C,C)
        np.concatenate([WT[6], WT[7]], 0),     # pass3  @+a  : (U,L)+(U,C)  (also corrA)
        np.concatenate([WT[2], WT[8]], 0),     # pass4  @-a B: (D,R)+(U,R)
        np.concatenate([WT[5], Z], 0),         # pass5  @ 0 B: (C,R)
        np.concatenate([WT[8], Z], 0),         # corrB       : (U,R) rows
    ]
    return np.concatenate(blocks, 1).astype(BF16)             # (128, 384)


# --------------------------------------------------------------------------
# device graph
# --------------------------------------------------------------------------

def _build_graph(caps, nchunks):
    key = (tuple(caps), nchunks)
    if key in _GRAPH_CACHE:
        return _GRAPH_CACHE[key]

    import concourse.bass as bass                 # noqa: F401
    import concourse.mybir as mybir
    import concourse.tile as tile
    from concourse import bacc

    S_pad = nchunks * 128
    bf16, i16, f32 = mybir.dt.bfloat16, mybir.dt.int16, mybir.dt.float32

    nc = bacc.Bacc("TRN2", target_bir_lowering=False, debug=False,
                   num_devices=NCORES, num_swdge_queues=4)
    tabA = nc.dram_tensor("tabA", [W * 3, ES], bf16, kind="ExternalInput")
    tabB = nc.dram_tensor("tabB", [W * 9, ES], bf16, kind="ExternalInput")
    idx_d = nc.dram_tensor("idx", [128, nchunks * 16], i16, kind="ExternalInput")
    wts_d = nc.dram_tensor("wts", [128, 6 * 64], bf16, kind="ExternalInput")
    bias_d = nc.dram_tensor("bias", [64, 1], f32, kind="ExternalInput")
    out_d = nc.dram_tensor("out", [nchunks, 128, H, 64], bf16,
                           kind="ExternalOutput")

    # block layout in global sorted coords
    starts = [0, caps[0], caps[0] + caps[1]]
    blocks = [(starts[i], starts[i] + caps[i], i + 1) for i in range(3)]

    with tile.TileContext(nc) as tc:
        with (
            tc.tile_pool(name="const", bufs=1) as constp,
            tc.tile_pool(name="slabA", bufs=3) as poolA,
            tc.tile_pool(name="slabB", bufs=3) as poolB,
            tc.tile_pool(name="stage", bufs=2) as stagep,
            tc.tile_pool(name="psum", bufs=6, space="PSUM") as psump,
            tc.tile_pool(name="psumc", bufs=2, space="PSUM") as psumcp,
        ):
            idx_sb = constp.tile([128, nchunks * 16], i16)
            nc.sync.dma_start(idx_sb[:], idx_d[:])
            wts_sb = constp.tile([128, 6 * 64], bf16)
            nc.sync.dma_start(wts_sb[:], wts_d[:])
            bias_sb = constp.tile([64, 1], f32)
            nc.sync.dma_start(bias_sb[:], bias_d[:])

            def lhs(p):
                return wts_sb[:, p * 64:(p + 1) * 64]

            evac_flip = [0]

            for cch in range(nchunks):
                k0g = cch * 128
                slabA = poolA.tile([128, HP, 128], bf16)
                nc.gpsimd.dma_gather(
                    out_ap=slabA[:], in_ap=tabA[:],
                    idxs_ap=idx_sb[:, cch * 16:cch * 16 + 8],
                    num_idxs=128, num_idxs_reg=128, elem_size=ES,
                    transpose=True, queue_num=(2 * cch) % 4)
                slabB = poolB.tile([128, HP, 128], bf16)
                nc.gpsimd.dma_gather(
                    out_ap=slabB[:], in_ap=tabB[:],
                    idxs_ap=idx_sb[:, cch * 16 + 8:cch * 16 + 16],
                    num_idxs=128, num_idxs_reg=128, elem_size=ES,
                    transpose=True, queue_num=(2 * cch + 1) % 4)
                # stage: partitions 0-63 = O for chunk cols 0-63,
                #        partitions 64-127 = O for chunk cols 64-127.
                # Consecutive windows alternate halves -> disjoint PE
                # column groups (concurrent M=64 matmuls).
                stage = stagep.tile([128, H, 64], bf16)
                S_tot = caps[0] + caps[1] + caps[2]
                if k0g + 128 > S_tot:
                    vlo = max(0, min(64, S_tot - k0g))
                    vhi = max(0, min(64, S_tot - k0g - 64))
                    if vlo < 64:
                        nc.vector.memset(stage[0:64, :, vlo:], 0.0)
                    if vhi < 64:
                        nc.vector.memset(stage[64:128, :, vhi:], 0.0)

                def pieces_in(r0, r1):
                    out = []
                    for (blo, bhi, a0) in blocks:
                        lo, hi = max(blo, k0g + r0), min(bhi, k0g + r1)
                        if lo < hi:
                            out.append((lo - k0g, hi - k0g, a0))
                    return out

                def windows_of(region_pieces):
                    w = []
                    for (lo, hi, a0) in region_pieces:
                        for wlo in range(lo, hi, KW):
                            w.append((wlo, min(KW, hi - wlo), a0))
                    return w

                lo_pieces = pieces_in(0, 64)
                up_pieces = pieces_in(64, 128)
                wlist_lo = windows_of(lo_pieces)
                wlist_up = windows_of(up_pieces)
                inter = []
                for i in range(max(len(wlist_lo), len(wlist_up))):
                    if i < len(wlist_lo):
                        inter.append((0,) + wlist_lo[i])
                    if i < len(wlist_up):
                        inter.append((1,) + wlist_up[i])

                for (half, wlo, kw, a0) in inter:
                    ps = psump.tile([128, H, kw], mybir.dt.float32, tag="ps")
                    pv = ps[64 * half:64 * half + 64]

                    def rhs(slab, d):
                        return slab[:, PADR + d:PADR + d + H, wlo:wlo + kw]

                    nc.tensor.matmul(pv, lhs(0), rhs(slabA, -a0),
                                     start=True, stop=False, skip_group_check=True)
                    nc.tensor.matmul(pv, lhs(1), rhs(slabA, 0),
                                     start=False, stop=False, skip_group_check=True)
                    nc.tensor.matmul(pv, lhs(2), rhs(slabA, +a0),
                                     start=False, stop=False, skip_group_check=True)
                    nc.tensor.matmul(pv, lhs(3), rhs(slabB, -a0),
                                     start=False, stop=False, skip_group_check=True)
                    nc.tensor.matmul(pv, lhs(4), rhs(slabB, 0),
                                     start=False, stop=True, skip_group_check=True)
                    dst = stage[64 * half:64 * half + 64, :,
                                wlo - 64 * half:wlo - 64 * half + kw]
                    if evac_flip[0] % 2 == 0:
                        nc.vector.tensor_scalar_add(dst, pv, bias_sb[:, :1])
                    else:
                        nc.scalar.activation(
                            dst, pv, mybir.ActivationFunctionType.Identity,
                            bias=bias_sb[:, :1])
                    evac_flip[0] += 1

                # top-edge clamp correction rows h >= H-a0
                for half, region in ((0, lo_pieces), (1, up_pieces)):
                    for (lo, hi, a0) in region:
                        n = hi - lo
                        psc = psumcp.tile([128, a0, n], mybir.dt.float32,
                                          tag="psc")
                        pcv = psc[64 * half:64 * half + 64]
                        rows = slice(PADR + H - a0, PADR + H)
                        nc.tensor.matmul(pcv, lhs(2), slabA[:, rows, lo:hi],
                                         start=True, stop=False, skip_group_check=True)
                        nc.tensor.matmul(pcv, lhs(5), slabB[:, rows, lo:hi],
                                         start=False, stop=True, skip_group_check=True)
                        cdst = stage[64 * half:64 * half + 64, H - a0:H,
                                     lo - 64 * half:hi - 64 * half]
                        nc.vector.tensor_tensor(
                            out=cdst, in0=pcv, in1=cdst,
                            op=mybir.AluOpType.add)

                nc.sync.dma_start(out_d[cch], stage[:])

    nc.compile()
    _GRAPH_CACHE[key] = nc
    return nc


def _register_ntff_hook():
    """Wire up NTFF profiling (image's antenv lacks the boot-time hook)."""
    try:
        import types
        import antenv
        if "antenv.axon_hooks" not in sys.modules:
            mod = types.ModuleType("antenv.axon_hooks")
            mod._HOOK = None

            def set_axon_ntff_profile_hook(hook):
                mod._HOOK = hook

            def get_axon_ntff_profile_hook():
                return mod._HOOK

            mod.set_axon_ntff_profile_hook = set_axon_ntff_profile_hook
            mod.get_axon_ntff_profile_hook = get_axon_ntff_profile_hook
            sys.modules["antenv.axon_hooks"] = mod
            antenv.axon_hooks = mod
        mod = sys.modules["antenv.axon_hooks"]
        if mod.get_axon_ntff_profile_hook() is None:
            from trn_agent_boot.trn_boot import _ntff_profile_via_ctypes
            hook = _ntff_profile_via_ctypes("/opt/axon/libaxon_pjrt.so")
            if hook is not None:
                mod.set_axon_ntff_profile_hook(hook)
    except Exception as e:          # profiling is best-effort
        print(f"ntff hook registration failed: {e}")


# --------------------------------------------------------------------------
# entry point
# --------------------------------------------------------------------------

def kernel(x, dh, dw, weight, bias, _sim=False, _trace=False):
    x = np.asarray(x, np.float32)
    dh = np.asarray(dh, np.float32)
    dw = np.asarray(dw, np.float32)
    weight = np.asarray(weight, np.float32)
    bias = np.asarray(bias, np.float32)

    ab_all = np.clip(dh[:, 0].astype(np.int64), 1, None)      # (B, W)
    eb_all = np.clip(dw[:, 0].astype(np.int64), 1, None)

    core_blocks = [_sort_info(ab_all[b], eb_all[b]) for b in range(B)]
    caps = [max(len(core_blocks[b][i]) for b in range(B)) for i in range(3)]
    S = sum(caps)
    nchunks = (S + 127) // 128

    nc = _build_graph(caps, nchunks)

    wts = _pack_weights(weight)
    bias_t = bias.reshape(64, 1).astype(np.float32)

    in_maps = []
    colmaps = []
    for b in range(B):
        tA, tB = _build_tables(x[b])
        idx, colmap = _pack_indices(core_blocks[b], eb_all[b], ab_all[b],
                                    caps, nchunks)
        colmaps.append(colmap)
        in_maps.append({"tabA": tA, "tabB": tB, "idx": idx,
                        "wts": wts, "bias": bias_t})

    if _sim:
        from concourse.bass_interp import CoreSim
        outs = []
        for b in range(B):
            sim = CoreSim(nc, core_id=0)
            for k, v in in_maps[b].items():
                sim.tensor(k)[:] = v
            sim.simulate()
            outs.append(np.array(sim.tensor("out")))
        results = [{"out": o} for o in outs]
        exec_ns = None
    else:
        from concourse.bass_utils import run_bass_kernel_spmd
        if _trace:
            _register_ntff_hook()
        r = run_bass_kernel_spmd(nc, in_maps, core_ids=list(range(NCORES)),
                                 trace=_trace)
        results = r.results
        exec_ns = r.exec_time_ns
        kernel.last_profile = r.profile_json

    out = np.zeros((B, O, H, W), np.float32)
    for b in range(B):
        ob = np.asarray(results[b]["out"]).astype(np.float32)
        ob = (ob.reshape(nchunks, 2, 64, H, 64)
                .transpose(2, 3, 0, 1, 4).reshape(64, H, nchunks * 128))
        for base, cols in colmaps[b]:
            out[b][:, :, cols] = ob[:, :, base:base + len(cols)]
    kernel.last_exec_ns = exec_ns
    return out


kernel.last_exec_ns = None
